# revision 10
# baseline (speedup 1.0000x reference)
"""Bass/Tile TRN2 kernel for nn_AttentionLayer (additive attention).

Reference computation (B=32, N=2048, E=D=H=1024):
    h      = tanh(enc @ w_e + (dec @ w_d)[:, None, :])   # [B, N, H]
    scores = (h @ w_out)[..., 0]                          # [B, N]
    scores = where(inp_mask, scores, -inf)
    probs  = softmax(scores, axis=-1)                     # [B, N]
    attn   = einsum('bn,bne->be', probs, enc)             # [B, E]
    return attn, probs

Distribution: data-parallel over batch; 8 NeuronCores x 4 batches each.
No collectives. Weights replicated.

Per-core dataflow (fp32 data, fp32r matmuls ~ 11-bit-mantissa operands):
  - enc rows stream in naturally ([row, E]); PE transpose-mode produces
    encT tiles [E-part, row-free] feeding the main matmul
    hT[h, row] = sum_e w_e[e, h] * encT[e, row]  (w_e natural = lhsT).
  - ACT applies tanh(x + dh[b, h]) straight out of PSUM with the
    per-partition bias dhT, producing fp32r hT tiles.
  - scores[1, row] accumulates over the 8 h-tiles with lhsT = w_out.
  - masking = add of (mask-1)*1e30; softmax on one partition.
  - probs transposed into [n-part, 1] columns via K=1 matmuls, then
    attn[1, E] accumulates probsT x enc_natural (second pass over enc,
    prefetched before the softmax since it does not depend on probs).
"""

import numpy as np

B, N, E, D, H = 32, 2048, 1024, 1024, 1024
NCORES = 8
BPC = B // NCORES          # batches per core = 4
RC = 512                   # row-chunk size
NCHUNK = N // RC           # 4 row chunks per batch

_COMPILED_NC = None


def _build_nc():
    from contextlib import ExitStack

    import concourse.tile as tile
    from concourse import bacc, mybir
    from concourse.masks import make_identity

    f32 = mybir.dt.float32
    f32r = mybir.dt.float32r
    u8 = mybir.dt.uint8
    Tanh = mybir.ActivationFunctionType.Tanh
    Exp = mybir.ActivationFunctionType.Exp

    nc = bacc.Bacc("TRN2", target_bir_lowering=False, debug=False)

    enc_l = nc.dram_tensor("enc_l", [BPC, N, E], f32, kind="ExternalInput").ap()
    dec_l = nc.dram_tensor("dec_l", [BPC, D], f32, kind="ExternalInput").ap()
    mask_l = nc.dram_tensor("mask_l", [BPC, N], u8, kind="ExternalInput").ap()
    w_e = nc.dram_tensor("w_e", [E, H], f32, kind="ExternalInput").ap()
    w_d = nc.dram_tensor("w_d", [D, H], f32, kind="ExternalInput").ap()
    w_out = nc.dram_tensor("w_out", [H, 1], f32, kind="ExternalInput").ap()
    attn_o = nc.dram_tensor("attn_o", [BPC, E], f32, kind="ExternalOutput").ap()
    probs_o = nc.dram_tensor("probs_o", [BPC, N], f32, kind="ExternalOutput").ap()

    KT = E // 128   # 8 k-tiles over E
    HT = H // 128   # 8 h-tiles over H
    NT = N // 128   # 16 n-tiles per batch

    with tile.TileContext(nc) as tc, ExitStack() as ctx:
        perm = ctx.enter_context(tc.tile_pool(name="perm", bufs=1))

        ident = perm.tile([128, 128], f32)
        make_identity(nc, ident)
        ones = perm.tile([1, 1], f32)
        nc.vector.memset(ones, 1.0)

        w_r = perm.tile([128, KT, H], f32r)        # w_e, [e-part, ec, h]
        wout_r = perm.tile([128, HT, 1], f32r)
        dhT = perm.tile([128, HT, BPC], f32)       # (dec @ w_d).T per h-tile
        dec_sb = perm.tile([BPC, D], f32)
        decT = perm.tile([128, KT, BPC], f32)
        wout_sb = perm.tile([128, HT, 1], f32)

        # ---- main pools ----
        nat = ctx.enter_context(tc.tile_pool(name="nat", bufs=8))
        encT = ctx.enter_context(tc.tile_pool(name="encT", bufs=10))
        hR = ctx.enter_context(tc.tile_pool(name="hR", bufs=10))
        sp = ctx.enter_context(tc.tile_pool(name="sp", bufs=2))
        small = ctx.enter_context(tc.tile_pool(name="small", bufs=2))
        ent2 = ctx.enter_context(tc.tile_pool(name="ent2", bufs=5))

        def load_chunk(b, rc):
            nts = []
            for rt in range(4):
                nt = nat.tile([128, E], f32, tag="nt")
                nc.sync.dma_start(
                    nt, enc_l[b, rc * RC + rt * 128: rc * RC + (rt + 1) * 128, :])
                nts.append(nt)
            return nts

        # prefetch the first chunk before the (DMA-heavy) weight loads
        prefetched = {(0, 0): load_chunk(0, 0)}

        # ---- setup: weights, dec transpose, dh = dec @ w_d ----
        nc.sync.dma_start(dec_sb, dec_l)
        nc.sync.dma_start(wout_sb, w_out.rearrange("(t p) o -> p t o", p=128))
        nc.vector.tensor_copy(wout_r, wout_sb)

        with tc.tile_pool(name="psA", bufs=2, space="PSUM") as psA:
            for kc in range(KT):
                pt = psA.tile([128, BPC], f32)
                nc.tensor.transpose(pt, dec_sb[:, kc * 128:(kc + 1) * 128],
                                    ident[0:BPC, 0:BPC])
                nc.scalar.copy(decT[:, kc, :], pt)

        we_rearr = w_e.rearrange("(ec p) h -> p ec h", p=128)
        for ec in range(KT):
            wec = nat.tile([128, H], f32, tag="nt")
            nc.sync.dma_start(wec, we_rearr[:, ec, :])
            nc.vector.tensor_copy(w_r[:, ec, :], wec)

        wd_rearr = w_d.rearrange("(kc p) h -> p kc h", p=128)
        with tc.tile_pool(name="psB", bufs=8, space="PSUM") as psB:
            pdhs = [psB.tile([128, BPC], f32, tag="pdh", name=f"pdh{i}") for i in range(HT)]
            for kc in range(KT):
                wdc = nat.tile([128, H], f32, tag="nt")
                nc.sync.dma_start(wdc, wd_rearr[:, kc, :])
                for ht in range(HT):
                    nc.tensor.matmul(pdhs[ht],
                                     wdc[:, ht * 128:(ht + 1) * 128],
                                     decT[:, kc, :],
                                     start=(kc == 0), stop=(kc == KT - 1))
            for ht in range(HT):
                nc.scalar.copy(dhT[:, ht, :], pdhs[ht])

        psT = ctx.enter_context(tc.tile_pool(name="psT", bufs=2, space="PSUM"))
        psH = ctx.enter_context(tc.tile_pool(name="psH", bufs=2, space="PSUM"))
        psS = ctx.enter_context(tc.tile_pool(name="psS", bufs=1, space="PSUM"))
        psPT = ctx.enter_context(tc.tile_pool(name="psPT", bufs=1, space="PSUM"))
        psAT = ctx.enter_context(tc.tile_pool(name="psAT", bufs=2, space="PSUM"))

        for b in range(BPC):
            scores_sb = sp.tile([1, N], f32, tag="scores")  # scores -> probs

            # ---------- phase 1: scores for batch b ----------
            for rc in range(NCHUNK):
                nts = prefetched.pop((b, rc), None) or load_chunk(b, rc)

                eTs = []
                for ec in range(KT):
                    ptile = psT.tile([128, RC], f32)
                    for rt in range(4):
                        nc.tensor.transpose(
                            ptile[:, rt * 128:(rt + 1) * 128],
                            nts[rt][:, ec * 128:(ec + 1) * 128],
                            ident)
                    eT = encT.tile([128, RC], f32r, tag="eT")
                    nc.vector.tensor_copy(eT, ptile)
                    eTs.append(eT)

                ps_score = psS.tile([1, RC], f32)
                for ht in range(HT):
                    ph = psH.tile([128, RC], f32)
                    for ec in range(KT):
                        nc.tensor.matmul(ph, w_r[:, ec, ht * 128:(ht + 1) * 128],
                                         eTs[ec], start=(ec == 0), stop=(ec == KT - 1))
                    h_t = hR.tile([128, RC], f32r, tag="h")
                    nc.scalar.activation(h_t, ph, Tanh, bias=dhT[:, ht, b:b + 1])
                    nc.tensor.matmul(ps_score, wout_r[:, ht, :], h_t,
                                     start=(ht == 0), stop=(ht == HT - 1))
                nc.vector.tensor_copy(scores_sb[:, rc * RC:(rc + 1) * RC], ps_score)

            # ---------- phase 2 prefetch: reload enc rows (no probs dep) ----
            ent_rs = []
            for kt in range(NT):
                ent = nat.tile([128, E], f32, tag="nt")
                nc.sync.dma_start(ent, enc_l[b, kt * 128:(kt + 1) * 128, :])
                ent_r = ent2.tile([128, E], f32r, tag="entr")
                if kt % 2 == 0:
                    nc.vector.tensor_copy(ent_r, ent)
                else:
                    nc.scalar.copy(ent_r, ent)
                ent_rs.append(ent_r)

            # ---------- softmax for batch b ----------
            # masking: scores += (mask-1)*1e30  (exact where mask==1)
            mrow_u8 = small.tile([1, N], u8, tag="msku")
            nc.sync.dma_start(mrow_u8, mask_l[b:b + 1, :])
            mrow = small.tile([1, N], f32, tag="mskf")
            nc.vector.tensor_scalar(mrow, mrow_u8, 1e30, -1e30,
                                    mybir.AluOpType.mult, mybir.AluOpType.add)
            nc.vector.tensor_add(scores_sb, scores_sb, mrow)
            mx = small.tile([1, 1], f32, tag="mx")
            nc.vector.reduce_max(mx, scores_sb, axis=mybir.AxisListType.X)
            negmx = small.tile([1, 1], f32, tag="negmx")
            nc.vector.tensor_scalar_mul(negmx, mx, -1.0)
            zsum = small.tile([1, 1], f32, tag="zsum")
            nc.scalar.activation(scores_sb, scores_sb, Exp, bias=negmx,
                                 accum_out=zsum)
            rz = small.tile([1, 1], f32, tag="rz")
            nc.vector.reciprocal(rz, zsum)
            nc.vector.tensor_scalar_mul(scores_sb, scores_sb, rz)  # now probs
            nc.sync.dma_start(probs_o[b:b + 1, :], scores_sb)

            # ---------- phase 2: attn for batch b ----------
            ps_pt = psPT.tile([128, NT], f32)
            for kt in range(NT):
                nc.tensor.matmul(ps_pt[:, kt:kt + 1],
                                 scores_sb[:, kt * 128:(kt + 1) * 128], ones,
                                 start=True, stop=True)
            pT_r = small.tile([128, NT], f32r, tag="pT")
            nc.vector.tensor_copy(pT_r, ps_pt)

            pa0 = psAT.tile([1, 512], f32, tag="pa")
            pa1 = psAT.tile([1, 512], f32, tag="pa")
            for kt in range(NT):
                nc.tensor.matmul(pa0, pT_r[:, kt:kt + 1], ent_rs[kt][:, 0:512],
                                 start=(kt == 0), stop=(kt == NT - 1))
                nc.tensor.matmul(pa1, pT_r[:, kt:kt + 1], ent_rs[kt][:, 512:1024],
                                 start=(kt == 0), stop=(kt == NT - 1))
            at_sb = small.tile([1, E], f32, tag="at")
            nc.scalar.copy(at_sb[:, 0:512], pa0)
            nc.scalar.copy(at_sb[:, 512:1024], pa1)
            nc.sync.dma_start(attn_o[b:b + 1, :], at_sb)

    nc.compile()
    return nc


def _get_nc():
    global _COMPILED_NC
    if _COMPILED_NC is None:
        _COMPILED_NC = _build_nc()
    return _COMPILED_NC


def kernel(enc, dec, inp_mask, w_e, w_d, w_out, _trace=False, _tmpdir=None):
    from concourse.bass_utils import run_bass_kernel_spmd

    enc = np.ascontiguousarray(np.asarray(enc, dtype=np.float32))
    dec = np.ascontiguousarray(np.asarray(dec, dtype=np.float32))
    mask = np.ascontiguousarray(np.asarray(inp_mask).astype(np.uint8))
    w_e = np.ascontiguousarray(np.asarray(w_e, dtype=np.float32))
    w_d = np.ascontiguousarray(np.asarray(w_d, dtype=np.float32))
    w_out = np.ascontiguousarray(np.asarray(w_out, dtype=np.float32))

    nc = _get_nc()
    in_maps = []
    for c in range(NCORES):
        sl = slice(c * BPC, (c + 1) * BPC)
        in_maps.append({
            "enc_l": enc[sl], "dec_l": dec[sl], "mask_l": mask[sl],
            "w_e": w_e, "w_d": w_d, "w_out": w_out,
        })
    kwargs = {}
    if _trace:
        kwargs.update(trace=True, tmpdir=_tmpdir)
    res = run_bass_kernel_spmd(nc, in_maps, core_ids=list(range(NCORES)), **kwargs)
    attn = np.concatenate([r["attn_o"] for r in res.results], axis=0)
    probs = np.concatenate([r["probs_o"] for r in res.results], axis=0)
    if _trace:
        return (attn, probs), res
    return attn, probs


# revision 11
# speedup vs baseline: 1.1536x; 1.1536x over previous
"""Bass/Tile TRN2 kernel for nn_AttentionLayer (additive attention).

Reference computation (B=32, N=2048, E=D=H=1024):
    h      = tanh(enc @ w_e + (dec @ w_d)[:, None, :])   # [B, N, H]
    scores = (h @ w_out)[..., 0]                          # [B, N]
    scores = where(inp_mask, scores, -inf)
    probs  = softmax(scores, axis=-1)                     # [B, N]
    attn   = einsum('bn,bne->be', probs, enc)             # [B, E]
    return attn, probs

Distribution: data-parallel over batch; 8 NeuronCores x 4 batches each.
No collectives. Weights replicated.

Per-core dataflow (fp32 data, fp32r matmuls ~ 11-bit-mantissa operands):
  - enc rows stream in naturally ([row, E]); PE transpose-mode produces
    encT tiles [E-part, row-free] feeding the main matmul
    hT[h, row] = sum_e w_e[e, h] * encT[e, row]  (w_e natural = lhsT).
  - ACT applies tanh(x + dh[b, h]) straight out of PSUM with the
    per-partition bias dhT, producing fp32r hT tiles.
  - scores[1, row] accumulates over the 8 h-tiles with lhsT = w_out.
  - masking = add of (mask-1)*1e30; softmax on one partition.
  - probs transposed into [n-part, 1] columns via K=1 matmuls, then
    attn[1, E] accumulates probsT x enc_natural (second pass over enc,
    prefetched before the softmax since it does not depend on probs).
"""

import numpy as np

B, N, E, D, H = 32, 2048, 1024, 1024, 1024
NCORES = 8
BPC = B // NCORES          # batches per core = 4
RC = 512                   # row-chunk size
NCHUNK = N // RC           # 4 row chunks per batch

_COMPILED_NC = None


def _build_nc():
    from contextlib import ExitStack

    import concourse.tile as tile
    from concourse import bacc, mybir
    from concourse.masks import make_identity

    f32 = mybir.dt.float32
    f32r = mybir.dt.float32r
    u8 = mybir.dt.uint8
    Tanh = mybir.ActivationFunctionType.Tanh
    Exp = mybir.ActivationFunctionType.Exp

    nc = bacc.Bacc("TRN2", target_bir_lowering=False, debug=False)

    enc_l = nc.dram_tensor("enc_l", [BPC, N, E], f32, kind="ExternalInput").ap()
    dec_l = nc.dram_tensor("dec_l", [BPC, D], f32, kind="ExternalInput").ap()
    mask_l = nc.dram_tensor("mask_l", [BPC, N], u8, kind="ExternalInput").ap()
    w_e = nc.dram_tensor("w_e", [E, H], f32, kind="ExternalInput").ap()
    w_d = nc.dram_tensor("w_d", [D, H], f32, kind="ExternalInput").ap()
    w_out = nc.dram_tensor("w_out", [H, 1], f32, kind="ExternalInput").ap()
    attn_o = nc.dram_tensor("attn_o", [BPC, E], f32, kind="ExternalOutput").ap()
    probs_o = nc.dram_tensor("probs_o", [BPC, N], f32, kind="ExternalOutput").ap()

    KT = E // 128   # 8 k-tiles over E
    HT = H // 128   # 8 h-tiles over H
    NT = N // 128   # 16 n-tiles per batch

    with tile.TileContext(nc) as tc, ExitStack() as ctx:
        perm = ctx.enter_context(tc.tile_pool(name="perm", bufs=1))

        ident = perm.tile([128, 128], f32)
        make_identity(nc, ident)
        ones = perm.tile([1, 1], f32)
        nc.vector.memset(ones, 1.0)

        w_r = perm.tile([128, KT, H], f32r)        # w_e, [e-part, ec, h]
        wout_r = perm.tile([128, HT, 1], f32r)
        dhT = perm.tile([128, HT, BPC], f32)       # (dec @ w_d).T per h-tile
        dec_sb = perm.tile([BPC, D], f32)
        decT = perm.tile([128, KT, BPC], f32)
        wout_sb = perm.tile([128, HT, 1], f32)

        # ---- main pools ----
        nat = ctx.enter_context(tc.tile_pool(name="nat", bufs=8))
        encT = ctx.enter_context(tc.tile_pool(name="encT", bufs=10))
        hR = ctx.enter_context(tc.tile_pool(name="hR", bufs=10))
        sp = ctx.enter_context(tc.tile_pool(name="sp", bufs=2))
        small = ctx.enter_context(tc.tile_pool(name="small", bufs=2))
        ent2 = ctx.enter_context(tc.tile_pool(name="ent2", bufs=4))
        nat2 = ctx.enter_context(tc.tile_pool(name="nat2", bufs=4))

        def load_chunk(b, rc):
            nts = []
            for rt in range(4):
                nt = nat.tile([128, E], f32, tag="nt")
                nc.sync.dma_start(
                    nt, enc_l[b, rc * RC + rt * 128: rc * RC + (rt + 1) * 128, :])
                nts.append(nt)
            return nts

        # prefetch the first chunk before the (DMA-heavy) weight loads
        prefetched = {(0, 0): load_chunk(0, 0)}

        # ---- setup: weights, dec transpose, dh = dec @ w_d ----
        nc.sync.dma_start(dec_sb, dec_l)
        nc.sync.dma_start(wout_sb, w_out.rearrange("(t p) o -> p t o", p=128))
        nc.vector.tensor_copy(wout_r, wout_sb)

        with tc.tile_pool(name="psA", bufs=2, space="PSUM") as psA:
            for kc in range(KT):
                pt = psA.tile([128, BPC], f32)
                nc.tensor.transpose(pt, dec_sb[:, kc * 128:(kc + 1) * 128],
                                    ident[0:BPC, 0:BPC])
                nc.scalar.copy(decT[:, kc, :], pt)

        we_rearr = w_e.rearrange("(ec p) h -> p ec h", p=128)
        for ec in range(KT):
            wec = nat2.tile([128, H], f32, tag="nt2")
            nc.sync.dma_start(wec, we_rearr[:, ec, :])
            nc.vector.tensor_copy(w_r[:, ec, :], wec)

        wd_rearr = w_d.rearrange("(kc p) h -> p kc h", p=128)
        with tc.tile_pool(name="psB", bufs=8, space="PSUM") as psB:
            pdhs = [psB.tile([128, BPC], f32, tag="pdh", name=f"pdh{i}") for i in range(HT)]
            for kc in range(KT):
                wdc = nat2.tile([128, H], f32, tag="nt2")
                nc.sync.dma_start(wdc, wd_rearr[:, kc, :])
                for ht in range(HT):
                    nc.tensor.matmul(pdhs[ht],
                                     wdc[:, ht * 128:(ht + 1) * 128],
                                     decT[:, kc, :],
                                     start=(kc == 0), stop=(kc == KT - 1))
            for ht in range(HT):
                nc.scalar.copy(dhT[:, ht, :], pdhs[ht])

        psT = ctx.enter_context(tc.tile_pool(name="psT", bufs=2, space="PSUM"))
        psH = ctx.enter_context(tc.tile_pool(name="psH", bufs=2, space="PSUM"))
        psS = ctx.enter_context(tc.tile_pool(name="psS", bufs=1, space="PSUM"))
        psPT = ctx.enter_context(tc.tile_pool(name="psPT", bufs=1, space="PSUM"))
        psAT = ctx.enter_context(tc.tile_pool(name="psAT", bufs=2, space="PSUM"))

        for b in range(BPC):
            scores_sb = sp.tile([1, N], f32, tag="scores")  # scores -> exp(s-mx)
            # masking terms: (mask-1)*1e30 (0 where valid, -1e30 where masked)
            mrow_u8 = small.tile([1, N], u8, tag="msku")
            nc.sync.dma_start(mrow_u8, mask_l[b:b + 1, :])
            mrow = small.tile([1, N], f32, tag="mskf")
            nc.vector.tensor_scalar(mrow, mrow_u8, 1e30, -1e30,
                                    mybir.AluOpType.mult, mybir.AluOpType.add)
            mx4 = small.tile([1, NCHUNK], f32, tag="mx4")

            # ---------- phase 1: scores for batch b ----------
            for rc in range(NCHUNK):
                nts = prefetched.pop((b, rc), None) or load_chunk(b, rc)

                eTs = []
                for ec in range(KT):
                    ptile = psT.tile([128, RC], f32)
                    for rt in range(4):
                        nc.tensor.transpose(
                            ptile[:, rt * 128:(rt + 1) * 128],
                            nts[rt][:, ec * 128:(ec + 1) * 128],
                            ident)
                    eT = encT.tile([128, RC], f32r, tag="eT")
                    nc.vector.tensor_copy(eT, ptile)
                    eTs.append(eT)

                ps_score = psS.tile([1, RC], f32)
                for ht in range(HT):
                    ph = psH.tile([128, RC], f32)
                    for ec in range(KT):
                        nc.tensor.matmul(ph, w_r[:, ec, ht * 128:(ht + 1) * 128],
                                         eTs[ec], start=(ec == 0), stop=(ec == KT - 1))
                    h_t = hR.tile([128, RC], f32r, tag="h")
                    nc.scalar.activation(h_t, ph, Tanh, bias=dhT[:, ht, b:b + 1])
                    nc.tensor.matmul(ps_score, wout_r[:, ht, :], h_t,
                                     start=(ht == 0), stop=(ht == HT - 1))
                csl = slice(rc * RC, (rc + 1) * RC)
                nc.vector.tensor_add(scores_sb[:, csl], ps_score, mrow[:, csl])
                nc.vector.reduce_max(mx4[:, rc:rc + 1], scores_sb[:, csl],
                                     axis=mybir.AxisListType.X)

            # ---------- phase 2 prefetch: reload enc rows (no probs dep) ----
            ent_rs = []
            for kt in range(NT):
                ent = nat2.tile([128, E], f32, tag="nt2")
                nc.sync.dma_start(ent, enc_l[b, kt * 128:(kt + 1) * 128, :])
                ent_r = ent2.tile([128, E], f32r, tag="entr")
                if kt % 2 == 0:
                    nc.vector.tensor_copy(ent_r, ent)
                else:
                    nc.scalar.copy(ent_r, ent)
                ent_rs.append(ent_r)

            # ---------- softmax for batch b ----------
            mx = small.tile([1, 1], f32, tag="mx")
            nc.vector.reduce_max(mx, mx4, axis=mybir.AxisListType.X)
            negmx = small.tile([1, 1], f32, tag="negmx")
            nc.vector.tensor_scalar_mul(negmx, mx, -1.0)
            zsum = small.tile([1, 1], f32, tag="zsum")
            nc.scalar.activation(scores_sb, scores_sb, Exp, bias=negmx,
                                 accum_out=zsum)  # unnormalized exp
            rz = small.tile([1, 1], f32, tag="rz")
            nc.vector.reciprocal(rz, zsum)

            # ---------- phase 2: attn for batch b (from unnormalized exp) ----
            ps_pt = psPT.tile([128, NT], f32)
            for kt in range(NT):
                nc.tensor.matmul(ps_pt[:, kt:kt + 1],
                                 scores_sb[:, kt * 128:(kt + 1) * 128], ones,
                                 start=True, stop=True)
            pT_r = small.tile([128, NT], f32r, tag="pT")
            nc.vector.tensor_copy(pT_r, ps_pt)

            # probs output = exp * (1/Z)  (runs off the critical path)
            probs_sb = small.tile([1, N], f32, tag="probs")
            nc.vector.tensor_scalar_mul(probs_sb, scores_sb, rz)
            nc.sync.dma_start(probs_o[b:b + 1, :], probs_sb)

            pa0 = psAT.tile([1, 512], f32, tag="pa")
            pa1 = psAT.tile([1, 512], f32, tag="pa")
            for kt in range(NT):
                nc.tensor.matmul(pa0, pT_r[:, kt:kt + 1], ent_rs[kt][:, 0:512],
                                 start=(kt == 0), stop=(kt == NT - 1))
                nc.tensor.matmul(pa1, pT_r[:, kt:kt + 1], ent_rs[kt][:, 512:1024],
                                 start=(kt == 0), stop=(kt == NT - 1))
            at_sb = small.tile([1, E], f32, tag="at")
            nc.scalar.activation(at_sb[:, 0:512], pa0,
                                 mybir.ActivationFunctionType.Copy, scale=rz)
            nc.scalar.activation(at_sb[:, 512:1024], pa1,
                                 mybir.ActivationFunctionType.Copy, scale=rz)
            nc.sync.dma_start(attn_o[b:b + 1, :], at_sb)

    nc.compile()
    return nc


def _get_nc():
    global _COMPILED_NC
    if _COMPILED_NC is None:
        _COMPILED_NC = _build_nc()
    return _COMPILED_NC


def kernel(enc, dec, inp_mask, w_e, w_d, w_out, _trace=False, _tmpdir=None):
    from concourse.bass_utils import run_bass_kernel_spmd

    enc = np.ascontiguousarray(np.asarray(enc, dtype=np.float32))
    dec = np.ascontiguousarray(np.asarray(dec, dtype=np.float32))
    mask = np.ascontiguousarray(np.asarray(inp_mask).astype(np.uint8))
    w_e = np.ascontiguousarray(np.asarray(w_e, dtype=np.float32))
    w_d = np.ascontiguousarray(np.asarray(w_d, dtype=np.float32))
    w_out = np.ascontiguousarray(np.asarray(w_out, dtype=np.float32))

    nc = _get_nc()
    in_maps = []
    for c in range(NCORES):
        sl = slice(c * BPC, (c + 1) * BPC)
        in_maps.append({
            "enc_l": enc[sl], "dec_l": dec[sl], "mask_l": mask[sl],
            "w_e": w_e, "w_d": w_d, "w_out": w_out,
        })
    kwargs = {}
    if _trace:
        kwargs.update(trace=True, tmpdir=_tmpdir)
    res = run_bass_kernel_spmd(nc, in_maps, core_ids=list(range(NCORES)), **kwargs)
    attn = np.concatenate([r["attn_o"] for r in res.results], axis=0)
    probs = np.concatenate([r["probs_o"] for r in res.results], axis=0)
    if _trace:
        return (attn, probs), res
    return attn, probs
